# revision 9
# baseline (speedup 1.0000x reference)
"""DenseCapsuleLayer forward on 8 Trainium2 NeuronCores — v2.1.

Sharding: in_num (i) split 8 ways (256 capsules/core); per-iteration
s-reduction is a [32,1024] bf16 AllReduce. All W/x data in bf16.

Per-core layout: partition p = 32*j + b with i_local = 4*q + j.
  U1 (bf16): [p, (q*1024 + o*32 + v)] holds u_hat.
Phase A: one fused DMA per 8-capsule chunk ([128, 1312] = W|xq|x2);
PE bf16 block-diagonal quad matmuls produce u_hat while s0 accumulates
from the x2 columns; PSUM->U1 copies split DVE/Act/Pool. AllReduce #1
fires right after s0's last matmul, overlapping the U1 tail.
Routing: c*u_hat and u_hat*v products are big DVE/Pool tensor_tensor ops;
i-reduction via PE 0/1-selection matmuls; v-reduction via pairwise bf16
folds; softmax runs per quad-range so s_pass starts before the straggler
engine finishes its logits. Logits f32.
"""
import sys
import numpy as np

sys.path.insert(0, "/opt/trn_rl_repo")

import concourse.bass as bass
import concourse.mybir as mybir
import concourse.tile as tile
from concourse.bass_utils import run_bass_kernel_spmd

F32 = mybir.dt.float32
BF16 = mybir.dt.bfloat16
ALU = mybir.AluOpType
ACTF = mybir.ActivationFunctionType
AXX = mybir.AxisListType.X

NCORES = 8
B, IN, D = 32, 2048, 16
O, V = 32, 32
IL = IN // NCORES          # 256 local input capsules
NQ = IL // 4               # 64 quads
NCH = IL // 8              # 32 W chunks
OV = O * V                 # 1024
CW = OV + 256 + 32         # fused chunk row: W | xq | x2
EPS = 1e-8

# engine split knobs
A_BLK = 26                 # a_pass max quads per block
S_BLK = 13                 # s_pass quads per chunk

_CACHE = {}


def split_multi_waits(nc, max_waits=1):
    """walrus CoreV3 rejects instructions carrying several semaphore waits;
    move extras onto same-engine NoOps inserted just before."""
    ctr = [0]

    def fresh_nop(engine, wait):
        ctr[0] += 1
        nop = mybir.InstNoOp(
            name=f"wsplit_nop_{ctr[0]}", ins=[], outs=[],
            sync_info=mybir.SyncInfo(on_wait=[wait], on_update=[]),
        )
        nop.engine = engine
        return nop

    for fn in [nc.main_func]:
        for bb in fn.blocks:
            insts = bb.instructions
            i = 0
            while i < len(insts):
                inst = insts[i]
                si = getattr(inst, "sync_info", None)
                if si is not None and si.on_wait and len(si.on_wait) > max_waits:
                    extra = list(si.on_wait[max_waits:])
                    si.on_wait[:] = list(si.on_wait[:max_waits])
                    nops = [fresh_nop(inst.engine, w) for w in extra]
                    for n in nops:
                        nc.register_instruction(n, overwrite=True)
                    insts[i:i] = nops
                    i += len(nops)
                i += 1
    return nc


def _ap4(tilehandle, dims, extra_off=0):
    """Custom AP on a tile: dims = [[stride, count], ...] after partitions."""
    base = tilehandle[:]
    part = list(base.ap[0])
    return bass.AP(base.tensor, base.offset + extra_off,
                   [part] + [list(d) for d in dims])


def _build_nc(do_ar=True, repeat_full=1, rep_a=1, rep_s=1, rep_sm=1):
    nc = bass.Bass("TRN2", target_bir_lowering=False, debug=False,
                   num_devices=NCORES)

    wx = nc.dram_tensor("wx", [NCH, 128, CW], BF16, kind="ExternalInput")
    sel = nc.dram_tensor("sel", [128, 32], BF16, kind="ExternalInput")
    y = nc.dram_tensor("y", [B, OV], BF16, kind="ExternalOutput")

    ar_in = nc.dram_tensor("ar_in", [B, OV], BF16)
    ar_out = nc.dram_tensor("ar_out", [B, OV], BF16, addr_space="Shared")

    with tile.TileContext(nc) as tc:
        with (
            tc.tile_pool(name="u1", bufs=1) as u1_pool,
            tc.tile_pool(name="wtile", bufs=2) as w_pool,
            tc.tile_pool(name="scr", bufs=1) as scr_pool,
            tc.tile_pool(name="big", bufs=1) as big_pool,
            tc.tile_pool(name="small", bufs=1) as small_pool,
            tc.tile_pool(name="psq", bufs=3, space="PSUM") as psq_pool,
            tc.tile_pool(name="pss", bufs=1, space="PSUM") as pss_pool,
        ):
            U1 = u1_pool.tile([128, NQ * OV], BF16)        # 128 KiB/part
            ts = scr_pool.tile([128, A_BLK * OV], BF16, tag="ts")
            sel_t = small_pool.tile([128, 32], BF16, tag="sel")
            nc.sync.dma_start(sel_t[:], sel.ap())

            a0f = big_pool.tile([128, NQ * O], F32, tag="a0")   # b1 logits
            a1f = big_pool.tile([128, NQ * O], F32, tag="a1")   # scratch / a1
            zz = small_pool.tile([128, NQ], F32, tag="zz")
            wrep = small_pool.tile([128, OV], BF16, tag="wrep")
            sgh = small_pool.tile([32, OV], BF16, tag="sgh")    # AR payload
            sq = small_pool.tile([32, O], F32, tag="sq")
            c1t = small_pool.tile([32, O], F32, tag="c1t")
            c2t = small_pool.tile([32, O], F32, tag="c2t")
            epsb = small_pool.tile([32, 1], F32, tag="epsb")
            zb = small_pool.tile([128, 1], F32, tag="zb")
            nc.vector.memset(epsb[:], EPS)
            nc.vector.memset(zb[:], 0.0)

            # ---------- Phase A: u_hat production + s0 ----------
            def copy_eng(c, jj):
                # Act takes 2/3 of the copies: cheaper per op there, and the
                # DVE queue drains sooner so AllReduce #1 can fire earlier.
                return nc.vector if (2 * c + jj) % 3 == 0 else nc.scalar

            def phase_a():
                s0ps = pss_pool.tile([32, OV], F32, tag="s0")
                xq_o, x2_o = OV, OV + 256
                for c in range(NCH):
                    wt = w_pool.tile([128, CW], BF16)
                    nc.sync.dma_start(wt[:], wx.ap()[c])
                    for h in range(2):
                        nc.tensor.matmul(
                            s0ps[:, h * 512:(h + 1) * 512],
                            wt[:, x2_o:x2_o + 32],
                            wt[:, h * 512:(h + 1) * 512],
                            start=(c == 0), stop=(c == NCH - 1),
                        )
                    for jj in range(2):
                        q = 2 * c + jj
                        qp = psq_pool.tile([128, OV], F32)
                        lhsT = wt[64 * jj:64 * (jj + 1),
                                  xq_o + 128 * jj:xq_o + 128 * (jj + 1)]
                        for h in range(2):
                            nc.tensor.matmul(
                                qp[:, h * 512:(h + 1) * 512],
                                lhsT,
                                wt[64 * jj:64 * (jj + 1), h * 512:(h + 1) * 512],
                                start=True, stop=True,
                            )
                        dst = U1[:, q * OV:(q + 1) * OV]
                        eng = copy_eng(c, jj)
                        if eng is nc.scalar:
                            eng.copy(dst, qp[:])
                        else:
                            eng.tensor_copy(dst, qp[:])
                return s0ps

            # ---------- helpers ----------
            def allreduce(src_ps):
                """PSUM -> sgh bf16 (DVE), HBM hop, AllReduce, back to sgh."""
                nc.vector.tensor_copy(sgh[:], src_ps[:])
                nc.sync.dma_start(ar_in.ap(), sgh[:])
                if do_ar:
                    nc.gpsimd.collective_compute(
                        "AllReduce", ALU.add,
                        replica_groups=[list(range(NCORES))],
                        ins=[ar_in.ap()], outs=[ar_out.ap()],
                    )
                    nc.sync.dma_start(sgh[:], ar_out.ap())
                else:
                    nc.sync.dma_start(sgh[:], ar_in.ap())

            def squash_to_w(scale, last=False):
                """sg = squash(sgh*scale); wrep = bf16 replica on 128 parts."""
                sqs = a1f[0:32, 0:OV]  # scratch (region dead at call sites)
                shv = sgh[:].rearrange("p (o v) -> p o v", o=O)
                nc.vector.tensor_tensor(
                    sqs.rearrange("p (o v) -> p o v", o=O), shv, shv,
                    op=ALU.mult)
                nc.vector.tensor_reduce(
                    sq[:], sqs.rearrange("p (o v) -> p o v", o=O),
                    axis=AXX, op=ALU.add)
                if scale != 1.0:
                    nc.vector.tensor_scalar_mul(sq[:], sq[:], scale * scale)
                nc.vector.tensor_scalar_add(c1t[:], sq[:], 1.0)
                nc.vector.reciprocal(c1t[:], c1t[:])
                nc.scalar.activation(c2t[:], sq[:], ACTF.Sqrt, bias=epsb[:32])
                nc.vector.reciprocal(c2t[:], c2t[:])
                nc.vector.tensor_tensor(c1t[:], c1t[:], sq[:], op=ALU.mult)
                nc.vector.tensor_tensor(c1t[:], c1t[:], c2t[:], op=ALU.mult)
                if scale != 1.0:
                    nc.vector.tensor_scalar_mul(c1t[:], c1t[:], scale)
                nc.vector.tensor_tensor(
                    shv, shv,
                    bass.AP(c1t.tensor, c1t[:].offset,
                            [list(c1t[:].ap[0]), [1, O], [0, V]]),
                    op=ALU.mult)
                if not last:
                    # broadcast v to all 128 partitions (SBUF->SBUF DMAs,
                    # spread over four queues so they land in parallel)
                    for j, eng in enumerate((nc.sync, nc.scalar, nc.gpsimd,
                                             nc.sync)):
                        eng.dma_start(wrep[32 * j:32 * (j + 1), :], sgh[:])

            def wrep_bcast(nq):
                return _ap4(wrep, [[0, nq], [V, O], [1, V]])

            def c_bcast(cf, q0, nq):
                return _ap4(cf, [[O, nq], [1, O], [0, V]], extra_off=q0 * O)

            def a_blocks(dst, q0=0, q1=NQ):
                eng = nc.vector
                while q0 < q1:
                    nq = min(A_BLK, q1 - q0)
                    t = ts[:, :nq * OV]
                    eng.tensor_tensor(
                        t.rearrange("p (q o v) -> p q o v", q=nq, o=O),
                        U1[:, q0 * OV:(q0 + nq) * OV]
                        .rearrange("p (q o v) -> p q o v", q=nq, o=O),
                        wrep_bcast(nq), op=ALU.mult)
                    w = V
                    while w > 2:
                        h = w // 2
                        tv = t.rearrange("p (f v) -> p f v", v=V)
                        eng.tensor_tensor(
                            tv[:, :, 0:h], tv[:, :, 0:h], tv[:, :, h:w],
                            op=ALU.add)
                        w = h
                    tv = t.rearrange("p (f v) -> p f v", v=V)
                    eng.tensor_tensor(
                        dst[:, q0 * O:(q0 + nq) * O]
                        .rearrange("p (f v) -> p f v", v=1),
                        tv[:, :, 0:1], tv[:, :, 1:2], op=ALU.add)
                    q0 += nq

            def softmax_full(logits, cdst):
                """cdst (f32) = softmax over o of logits; no max-shift
                (|logits| stays far below f32 exp overflow)."""
                cv = cdst[:].rearrange("p (q o) -> p q o", q=NQ)
                nc.scalar.activation(cdst[:], logits[:], ACTF.Exp,
                                     bias=zb[:])
                nc.vector.tensor_reduce(zz[:], cv, axis=AXX, op=ALU.add)
                nc.vector.reciprocal(zz[:], zz[:])
                nc.vector.tensor_tensor(
                    cv, cv, _ap4(zz, [[1, NQ], [0, O]]), op=ALU.mult)

            def s_pass(cf):
                """PSUM [32, OV] = sum_i c * u_hat (PE sel matmuls)."""
                sps = pss_pool.tile([32, OV], F32, tag="s0")
                sizes = [13, 13, 13, 13, 8, 4]
                chunks = []
                q0 = 0
                for nq in sizes:
                    chunks.append((q0, nq))
                    q0 += nq
                assert q0 == NQ
                for k, (q0, nq) in enumerate(chunks):
                    off = (k % 2) * S_BLK * OV
                    t = ts[:, off:off + nq * OV]
                    nc.vector.tensor_tensor(
                        t.rearrange("p (q o v) -> p q o v", q=nq, o=O),
                        U1[:, q0 * OV:(q0 + nq) * OV]
                        .rearrange("p (q o v) -> p q o v", q=nq, o=O),
                        c_bcast(cf, q0, nq), op=ALU.mult)
                    for qq in range(nq):
                        for h in range(2):
                            nc.tensor.matmul(
                                sps[:, h * 512:(h + 1) * 512],
                                sel_t[:],
                                t[:, qq * OV + h * 512:qq * OV + (h + 1) * 512],
                                start=(k == 0 and qq == 0),
                                stop=(k == len(chunks) - 1 and qq == nq - 1),
                            )
                return sps

            # ---------- main ----------
            for _full in range(repeat_full):
                s0ps = phase_a()
                allreduce(s0ps)            # overlaps U1-copy tail
                squash_to_w(1.0 / O)
                for _ra in range(rep_a):
                    a_blocks(a0f)
                for _rsm in range(rep_sm):
                    softmax_full(a0f, a1f)

                for _rs in range(rep_s):
                    s1 = s_pass(a1f)
                allreduce(s1)
                squash_to_w(1.0)
                for _ra in range(rep_a):
                    a_blocks(a1f)
                nc.vector.tensor_tensor(a1f[:], a1f[:], a0f[:], op=ALU.add)
                softmax_full(a1f, a0f)

                for _rs in range(rep_s):
                    s2 = s_pass(a0f)
                allreduce(s2)
                squash_to_w(1.0, last=True)
                if _full == 0:
                    nc.sync.dma_start(y.ap(), sgh[:])

    split_multi_waits(nc)
    return nc


def _prep_core(x, W, k):
    import ml_dtypes
    BFn = ml_dtypes.bfloat16
    xk = x[:, k * IL:(k + 1) * IL, :]                    # [B, IL, D]
    Wk = W[:, k * IL:(k + 1) * IL, :, :]                 # [O, IL, V, D]
    xt = np.ascontiguousarray(xk.transpose(1, 2, 0))     # [IL, D, B]
    Wt = np.ascontiguousarray(Wk.transpose(1, 3, 0, 2))  # [IL, D, O, V]
    wx = np.zeros((NCH, 128, CW), BFn)
    wx[:, :, :OV] = Wt.reshape(NCH, 128, OV)
    xtc = xt.reshape(NCH, 8, D, B)
    for jj in range(2):
        for j in range(4):
            r = 4 * jj + j
            wx[:, 64 * jj + 16 * j:64 * jj + 16 * (j + 1),
               OV + 128 * jj + 32 * j:OV + 128 * jj + 32 * (j + 1)] = xtc[:, r]
    wx[:, :, OV + 256:] = xt.reshape(NCH, 128, B)
    return {"wx": wx}


def _sel_np():
    E = np.zeros((128, 32), np.float32)
    for j in range(4):
        E[32 * j + np.arange(32), np.arange(32)] = 1.0
    return E


def kernel(x: np.ndarray, W: np.ndarray) -> np.ndarray:
    x = np.asarray(x, np.float32)
    W = np.asarray(W, np.float32)
    if "nc" not in _CACHE:
        _CACHE["nc"] = _build_nc()
    nc = _CACHE["nc"]

    import ml_dtypes
    sel = _sel_np().astype(ml_dtypes.bfloat16)
    in_maps = []
    for k in range(NCORES):
        m = _prep_core(x, W, k)
        m["sel"] = sel
        in_maps.append(m)
    res = run_bass_kernel_spmd(nc, in_maps, list(range(NCORES)))
    out = res.results[0]["y"].reshape(B, O, V)
    return np.ascontiguousarray(out.astype(np.float32))
